# revision 13
# baseline (speedup 1.0000x reference)
"""Trainium2 Bass kernel for nn_Boftrainer_48284022342309 (vq_codebook).

Data-parallel over batch: 64 images -> 8 cores x 8 images each.

Layout conventions (chosen so every SB->SB rhs-build DMA moves
partition-contiguous blocks):
  activation partitions: (px outer, channel inner):  p = px*C + c
  conv rhs partitions:   (j outer, channel inner):   p = j*C + c
  second image of a pair lives at partition offset 64.

Per-core pipeline (everything stays on-chip after the initial loads):
  warmup PE p-state ramp: ~10 dummy matmuls during the input load phase
         bring the tensor engine to full clock before conv1 starts.
  conv1  host-im2col rhs [54=(c,dy,j), (b,y,xb)] fp16; one matmul per
         (img, y-half), M=64=(px4,o16) multipixel columns.
  conv2  rhs [96=(j6,c16), (y66,xb16)] via contiguous SB->SB DMAs;
         3 dy-accumulated matmuls per image, image pair packed in PSUM.
  pool   pool-y: tensor_tensor max on free-strided PSUM views;
         pool-x: stream_shuffle(group half-swap) + fused relu/max.
  conv3  rhs [64=(j4,c16), (Y34,xb16)]; 3 dy matmuls M=48=(px2,o24).
  conv4  rhs [96=(j4,c24), (y34,xb16)]; px-split matmuls M=16 put each
         (img,px) f-slot at a 32-aligned partition base.
  RBF    f' = [f(16); f2; 1] rows per slot; four fp16 matmuls per slot
         against the sigma-augmented codebook fill a 4-bank PSUM strip
         with -sigma*dist2; ONE wide ACT Exp (2048 elems) produces a;
         per-pixel sums S on DVE (reduce over the K axis); reciprocal;
         pooled rows accumulate into a single PSUM bank with image i's
         pooled vector landing on PSUM row i (reciprocal placed at lhsT
         column i).
  MLP    pooled rows 0..8 cast once to SBUF, transposed to [128,(q,b)]
         via PE; two small matmuls; biases via per-partition ACT bias.
"""
import sys
sys.path.insert(0, "/opt/trn_rl_repo")

import contextlib

import numpy as np

import concourse.bass as bass
import concourse.tile as tile
from concourse import bacc, mybir
from concourse import bass_utils

F32 = mybir.dt.float32
F16 = mybir.dt.float16

N_CORES = 8
B_CORE = 8
H = W = 64
HP = 32
KCB = 512

AF = mybir.ActivationFunctionType
ALU = mybir.AluOpType

# column map of the packed fp16 constant buffer [128, CPACK_COLS]
_C_W1P = 0            # [54, 64]
_C_W2P = 64           # [96, 3*64]
_C_W3P = 256          # [64, 3*64]
_C_W4P0 = 448         # [97, 3*32]
_C_W4P1 = 544         # [97, 3*32]
_C_CAUG = 640         # [128, 512]
_C_BLOCKE = 1152      # [128, 128]
_C_M1W = 1280         # [128, 4*20]
_C_M2W = 1360         # [20, 10]
_C_ID8 = 1370         # [8, 8]
CPACK_COLS = 1378


# --------------------------------------------------------------------------
# host-side transforms (numpy)
# --------------------------------------------------------------------------

def _conv_weight_mp(w, P, J):
    """w [O,C,3,3] -> per-dy lhsT [J*C, P*O]:
    W[dy][j*C+c, px*O+o] = w[o,c,dy,j-px] if 0<=j-px<=2 else 0."""
    O, C = w.shape[0], w.shape[1]
    out = np.zeros((3, J * C, P * O), np.float32)
    for dy in range(3):
        for j in range(J):
            for px in range(P):
                dx = j - px
                if 0 <= dx <= 2:
                    out[dy, j * C:(j + 1) * C, px * O:(px + 1) * O] = \
                        w[:, :, dy, dx].T
    return out


def _prep_weights(w1, w2, w3, w4, codebook, sigma, l1_w, l1_b, l2_w, l2_b):
    cp = np.zeros((128, CPACK_COLS), np.float32)
    # conv1: full (c,dy,j) contraction in one matmul: [54, 64=(px4,o16)]
    W1p = np.zeros((3, 3, 6, 4, 16), np.float32)  # c dy j px o
    for j in range(6):
        for px in range(4):
            dx = j - px
            if 0 <= dx <= 2:
                W1p[:, :, j, px, :] = w1[:, :, :, dx].transpose(1, 2, 0)
    cp[0:54, _C_W1P:_C_W1P + 64] = W1p.reshape(54, 64)

    w2p = _conv_weight_mp(w2, P=4, J=6).transpose(1, 0, 2)   # [96,3,64]
    cp[0:96, _C_W2P:_C_W2P + 192] = w2p.reshape(96, 192)
    w3p = _conv_weight_mp(w3, P=2, J=4)                       # [3,64,48]
    w3pad = np.zeros((3, 64, 64), np.float32)
    w3pad[:, :, 0:48] = w3p
    cp[0:64, _C_W3P:_C_W3P + 192] = w3pad.transpose(1, 0, 2).reshape(64, 192)
    w4p = _conv_weight_mp(w4, P=2, J=4)                       # [3,96,32]
    # [97,...]: row 96 pairs with the constant-1 row of rhs4 and writes
    # 1.0 into ps4 row 32s+17 (the "ones" aug row) at dy=1.
    for px in (0, 1):
        wp = np.zeros((97, 3, 32), np.float32)
        wp[0:96, :, 0:16] = w4p[:, :, 16 * px:16 * px + 16].transpose(1, 0, 2)
        wp[96, 1, 17] = 1.0
        base = _C_W4P0 if px == 0 else _C_W4P1
        cp[0:97, base:base + 96] = wp.reshape(97, 96)

    c2 = (codebook * codebook).sum(axis=1)
    caug = np.zeros((32, KCB), np.float32)
    caug[0:16] = (2.0 * sigma[:, None] * codebook).T
    caug[16] = -sigma
    caug[17] = -sigma * c2
    cp[:, _C_CAUG:_C_CAUG + KCB] = np.tile(caug, (4, 1))

    be = np.zeros((128, 128), np.float32)
    for s in range(4):
        be[32 * s:32 * s + 16, 32 * s + 16] = 1.0
    cp[:, _C_BLOCKE:_C_BLOCKE + 128] = be

    l1s = l1_w / float(HP * HP)                               # [20, 512]
    m1w = np.ascontiguousarray(l1s.T).reshape(4, 128, 20)     # (q p) o
    cp[:, _C_M1W:_C_M1W + 80] = m1w.transpose(1, 0, 2).reshape(128, 80)
    cp[0:20, _C_M2W:_C_M2W + 10] = np.ascontiguousarray(l2_w.T)
    cp[0:8, _C_ID8:_C_ID8 + 8] = np.eye(8, dtype=np.float32)

    return {
        "cpack": cp.astype(np.float16),
        "m1b": l1_b.reshape(20, 1).astype(np.float32),
        "m2b": l2_b.reshape(10, 1).astype(np.float32),
    }


def _prep_rhs1(x):
    """x [B,3,64,64] -> [54=(c,dy,j), B*64*16] fp32.
    rhs1[(c,dy,j),(b,y,xb)] = xpad[b, c, y+dy, 4*xb+j] (xpad has 1-px halo)."""
    B = x.shape[0]
    xpad = np.zeros((B, 3, H + 2, W + 2), np.float32)
    xpad[:, :, 1:-1, 1:-1] = x
    rhs1 = np.empty((3, 3, 6, B, H, 16), np.float32)
    xb_idx = 4 * np.arange(16)
    for dy in range(3):
        for j in range(6):
            rhs1[:, dy, j] = xpad[:, :, dy:dy + H, :][:, :, :, xb_idx + j] \
                .transpose(1, 0, 2, 3)
    return rhs1.reshape(54, B * H * 16)


# --------------------------------------------------------------------------
# device kernel
# --------------------------------------------------------------------------

def build_kernel(tc, outs, ins):
    nc = tc.nc
    with contextlib.ExitStack() as ctx:
        consts = ctx.enter_context(tc.tile_pool(name="consts", bufs=1))
        acts = ctx.enter_context(tc.tile_pool(name="acts", bufs=1))
        work = ctx.enter_context(tc.tile_pool(name="work", bufs=3))
        rbf = ctx.enter_context(tc.tile_pool(name="rbf", bufs=2))
        pc = ctx.enter_context(tc.tile_pool(name="pc", bufs=3, space="PSUM"))
        pa = ctx.enter_context(tc.tile_pool(name="pa", bufs=4, space="PSUM"))
        pp = ctx.enter_context(tc.tile_pool(name="pp", bufs=1, space="PSUM"))

        # ---- constants: one big fp16 pack + two tiny f32 biases ----
        cpack = consts.tile([128, CPACK_COLS], F16, tag="cpack", name="cpack")
        nc.sync.dma_start(cpack[:], ins["cpack"][:])
        m1b = consts.tile([20, 1], F32, tag="m1b", name="m1b")
        nc.scalar.dma_start(m1b[:], ins["m1b"][:])
        m2b = consts.tile([10, 1], F32, tag="m2b", name="m2b")
        nc.scalar.dma_start(m2b[:], ins["m2b"][:])

        w1p = cpack[0:54, _C_W1P:_C_W1P + 64]
        w2p = cpack[0:96, _C_W2P:_C_W2P + 192].rearrange(
            "p (d m) -> p d m", d=3)
        w3p = cpack[0:64, _C_W3P:_C_W3P + 192].rearrange(
            "p (d m) -> p d m", d=3)
        w4p0 = cpack[0:97, _C_W4P0:_C_W4P0 + 96].rearrange(
            "p (d m) -> p d m", d=3)
        w4p1 = cpack[0:97, _C_W4P1:_C_W4P1 + 96].rearrange(
            "p (d m) -> p d m", d=3)
        caug = cpack[:, _C_CAUG:_C_CAUG + KCB]
        blocke = cpack[:, _C_BLOCKE:_C_BLOCKE + 128]
        m1w = cpack[:, _C_M1W:_C_M1W + 80].rearrange("p (q o) -> p q o", q=4)
        m2w = cpack[0:20, _C_M2W:_C_M2W + 10]
        ident8 = cpack[0:8, _C_ID8:_C_ID8 + 8]

        # ---- input: im2col'd rhs1, 4 chunks on different queues ----
        rhs1 = consts.tile([54, B_CORE, H, 16], F16, tag="rhs1", name="rhs1")
        r1src = ins["rhs1"].rearrange("k (b y xb) -> k b y xb",
                                      b=B_CORE, y=H, xb=16)
        r1eng = [nc.gpsimd, nc.scalar, nc.sync, nc.gpsimd]
        for c in range(4):
            r1eng[c].dma_start(rhs1[:, 2 * c:2 * c + 2], r1src[:, 2 * c:2 * c + 2])

        # ---- PE p-state warm-up: dependency-free dummy matmuls across 4
        # rotating PSUM banks during the load phase (same conditions as the
        # microbenchmark that reaches the full 2.4 GHz p-state).
        wsrc = consts.tile([128, 512], F16, tag="wsrc", name="wsrc")
        nc.vector.memset(wsrc[:], 0.25)
        warm_banks = [pa.tile([128, 512], F32, tag="pa", name=f"warm{_i}")
                      for _i in range(4)]
        for k in range(14):
            nc.tensor.matmul(warm_banks[k % 4][:], wsrc[:, 0:128], wsrc[:],
                             start=True, stop=True)

        def heartbeat(k):
            # one warm matmul to keep the PE activity monitor from
            # re-throttling across a known multi-us PE-idle stretch
            hb = pa.tile([128, 512], F32, tag="pa", name=f"hb{k}")
            nc.tensor.matmul(hb[:], wsrc[:, 0:128], wsrc[:],
                             start=True, stop=True)

        # round-robin DMA queue assignment for the conv rhs builds
        dma_engs = [nc.sync, nc.scalar, nc.gpsimd]
        _dma_i = [0]

        def next_eng():
            e = dma_engs[_dma_i[0] % len(dma_engs)]
            _dma_i[0] += 1
            return e

        # ---- persistent activation buffers ----
        act1 = [acts.tile([128, H, 16], F16, tag=f"act1_{p}", name=f"act1_{p}")
                for p in range(4)]
        rhs2 = [acts.tile([96, H + 2, 16], F16, tag=f"rhs2_{i}", name=f"rhs2_{i}")
                for i in range(4)]
        act2p = [acts.tile([128, HP + 2, 16], F16, tag=f"act2p_{p}",
                           name=f"act2p_{p}") for p in range(4)]
        rhs3 = [acts.tile([64, HP + 2, 16], F16, tag=f"rhs3_{i}", name=f"rhs3_{i}")
                for i in range(4)]
        act3 = [acts.tile([128, HP + 2, 16], F16, tag=f"act3_{p}",
                          name=f"act3_{p}") for p in range(4)]
        rhs4 = [acts.tile([97, HP + 2, 16], F16, tag=f"rhs4_{i}", name=f"rhs4_{i}")
                for i in range(4)]
        f_buf = [acts.tile([128, 512], F16, tag=f"f_{t}", name=f"f_{t}")
                 for t in range(4)]
        mlp_rhs = acts.tile([128, 4, B_CORE], F16, tag="mlp_rhs")

        for r2 in rhs2:
            nc.vector.memset(r2[:, 0:1, :], 0.0)
            nc.vector.memset(r2[:, 65:66, :], 0.0)
            nc.gpsimd.memset(r2[0:16, :, 0:1], 0.0)
            nc.gpsimd.memset(r2[64:96, :, 15:16], 0.0)
        for r3 in rhs3:
            nc.gpsimd.memset(r3[0:16, :, 0:1], 0.0)
            nc.gpsimd.memset(r3[32:64, :, 15:16], 0.0)
        for r4 in rhs4:
            nc.gpsimd.memset(r4[0:24, :, 0:1], 0.0)
            nc.gpsimd.memset(r4[64:96, :, 15:16], 0.0)
            nc.gpsimd.memset(r4[96:97, :, :], 1.0)
        for a2 in act2p:
            nc.vector.memset(a2[:, 0:1, :], 0.0)
            nc.vector.memset(a2[:, 33:34, :], 0.0)
        for a3 in act3:
            nc.gpsimd.memset(a3[:, 0:1, :], 0.0)
            nc.gpsimd.memset(a3[:, 33:34, :], 0.0)

        # ================= conv1 =================
        # act1 partition = 64*half + px*16 + o
        for pair in range(4):
            bA, bB = 2 * pair, 2 * pair + 1
            for h in range(2):
                ps = pc.tile([128, 32, 16], F32, tag="psc")
                nc.tensor.matmul(ps[0:64], w1p,
                                 rhs1[:, bA, 32 * h:32 * h + 32, :],
                                 start=True, stop=True)
                nc.tensor.matmul(ps[64:128], w1p,
                                 rhs1[:, bB, 32 * h:32 * h + 32, :],
                                 start=True, stop=True)
                # relu on ACT: idle during the conv phase, frees DVE
                nc.scalar.activation(
                    act1[pair][:, 32 * h:32 * h + 32, :], ps[:], AF.Relu)

        # ================= conv2 + pool =================
        # j -> (source px, xb shift): x = 4*xb_dst + j - 1 = 4*xb_src + px
        J2 = [(3, -1), (0, 0), (1, 0), (2, 0), (3, 0), (0, 1)]
        HSWAP = [(i + 16) % 32 for i in range(32)]  # swap px-pair halves
        for pair in range(4):
            for half in range(2):
                ioff = 64 * half
                r2 = rhs2[2 * (pair % 2) + half]
                # j=1..4 read contiguous px-blocks 0..3 -> one DMA
                next_eng().dma_start(r2[16:80, 1:65, :],
                                     act1[pair][ioff:ioff + 64, :, :])
                for j in (0, 5):
                    pj, sh = J2[j]
                    n = 16 - abs(sh)
                    d0, s0 = max(0, -sh), max(0, sh)
                    next_eng().dma_start(
                        r2[16 * j:16 * j + 16, 1:65, d0:d0 + n],
                        act1[pair][ioff + 16 * pj:ioff + 16 * pj + 16, :,
                                   s0:s0 + n])
            for h in range(2):
                ps = pc.tile([128, 32, 16], F32, tag="psc")
                for half in range(2):
                    for dy in range(3):
                        nc.tensor.matmul(
                            ps[64 * half:64 * half + 64],
                            w2p[:, dy, :],
                            rhs2[2 * (pair % 2) + half][:, 32 * h + dy:32 * h + dy + 32, :],
                            start=(dy == 0), stop=(dy == 2),
                            tile_position=(0, 64 * half))
                # relu (psum fp32 -> sbuf fp16) on ACT, then pool on DVE
                t0 = work.tile([128, 32, 16], F16, tag="t0")
                nc.scalar.activation(t0[:], ps[:], AF.Relu)
                tp = work.tile([128, 16, 16], F16, tag="tp")
                v = t0[:].rearrange("p (Y yp) x -> p Y yp x", yp=2)
                nc.vector.tensor_tensor(tp[:], v[:, :, 0, :], v[:, :, 1, :],
                                        op=ALU.max)
                # pool-x: swap 16-blocks within 32-groups
                sh_t = work.tile([128, 16, 16], F16, tag="sh")
                nc.vector.stream_shuffle(sh_t[:], tp[:], HSWAP)
                nc.vector.tensor_tensor(
                    act2p[pair][:, 1 + 16 * h:17 + 16 * h, :],
                    sh_t[:], tp[:], op=ALU.max)
            heartbeat(pair)

        # ================= conv3 =================
        # x3 = 2*xb3 + j - 1; source px-representative block in {0, 2}
        J3 = [(2, -1), (0, 0), (2, 0), (0, 1)]
        for pair in range(4):
            for half in range(2):
                ioff = 64 * half
                r3 = rhs3[2 * (pair % 2) + half]
                for j in (1, 2):
                    pj, sh = J3[j]
                    next_eng().dma_start(
                        r3[16 * j:16 * j + 16, :, :],
                        act2p[pair][ioff + 16 * pj:ioff + 16 * pj + 16, :, :])
                for j in (0, 3):
                    pj, sh = J3[j]
                    n = 16 - abs(sh)
                    d0, s0 = max(0, -sh), max(0, sh)
                    next_eng().dma_start(
                        r3[16 * j:16 * j + 16, :, d0:d0 + n],
                        act2p[pair][ioff + 16 * pj:ioff + 16 * pj + 16, :,
                                    s0:s0 + n])
            ps = pc.tile([128, 32, 16], F32, tag="psc")
            for half in range(2):
                for dy in range(3):
                    nc.tensor.matmul(
                        ps[64 * half:64 * half + 64],
                        w3p[:, dy, :],
                        rhs3[2 * (pair % 2) + half][:, dy:dy + 32, :],
                        start=(dy == 0), stop=(dy == 2),
                        tile_position=(0, 64 * half))
            nc.vector.tensor_scalar_max(act3[pair][:, 1:33, :], ps[:], 0.0)
            heartbeat(4 + pair)

        # ================= conv4 + f' assembly + RBF =================
        # act3 partition = 64*half + px*24 + o (px in {0,1})
        # RBF is interleaved per image-pair t so the ACT-bound exp work of
        # pair t overlaps the PE/DVE-bound conv4 work of pair t+1.
        # pooled vectors for all 8 images accumulate into ONE psum bank;
        # image i's pooled vector lands on psum row i because its 1/S
        # values sit at lhsT column i.
        J4 = [(1, -1), (0, 0), (1, 0), (0, 1)]
        ppool = pp.tile([128, 512], F32, tag="ppool", name="ppool")
        n_pool_mm = [0]

        for t in range(4):
            fb = f_buf[t]
            ps4 = pc.tile([128, 512], F32, tag="psc", name=f"ps4_{t}")
            for i in range(2):
                img = 2 * t + i
                pair, half = img // 2, img % 2
                ioff = 64 * half
                r4 = rhs4[2 * (t % 2) + i]
                # j=1,2 read contiguous px-blocks 0,1 -> one DMA
                next_eng().dma_start(r4[24:72, :, :],
                                     act3[pair][ioff:ioff + 48, :, :])
                for j in (0, 3):
                    pj, sh = J4[j]
                    n = 16 - abs(sh)
                    d0, s0 = max(0, -sh), max(0, sh)
                    next_eng().dma_start(
                        r4[24 * j:24 * j + 24, :, d0:d0 + n],
                        act3[pair][ioff + 24 * pj:ioff + 24 * pj + 24, :,
                                   s0:s0 + n])
                for px in range(2):
                    s = 2 * i + px
                    w4 = w4p0 if px == 0 else w4p1
                    for dy in range(3):
                        nc.tensor.matmul(
                            ps4[32 * s:32 * s + 32, :],
                            w4[:, dy, :],
                            r4[:, dy:dy + 32, :].rearrange("p y x -> p (y x)"),
                            start=(dy == 0), stop=(dy == 2),
                            tile_position=(0, 32 * s))
            nc.vector.tensor_scalar_max(fb[:], ps4[:], 0.0)
            fsq = work.tile([128, 512], F16, tag="fsq")
            nc.vector.tensor_mul(fsq[:], fb[:], fb[:])
            psf = pc.tile([128, 512], F32, tag="psc", name=f"psf_{t}")
            nc.tensor.matmul(psf[:], blocke, fsq[:], start=True, stop=True)
            nc.vector.tensor_tensor(fb[:], fb[:], psf[:], op=ALU.max)

            # ---- RBF for this image pair ----
            # q-major emission: the 4 slots' psa matmuls of each q batch
            # target distinct 32-row groups and distinct PSUM banks, so the
            # PE runs them nearly concurrently while ACT drains the exps.
            a_buf = [rbf.tile([128, 4, 512], F16, tag="a", name=f"a_{t}_{s}",
                              bufs=8) for s in range(4)]
            S_sl = [rbf.tile([128, 4], F32, tag="S", name=f"S_{t}_{s}",
                             bufs=8) for s in range(4)]
            for q in range(4):
                for s in range(4):
                    psa = pa.tile([128, 512], F32, tag="pa")
                    nc.tensor.matmul(
                        psa[:],
                        fb[32 * s:32 * s + 18, 128 * q:128 * q + 128],
                        caug[32 * s:32 * s + 18, :],
                        start=True, stop=True,
                        tile_position=(32 * s, 0))
                    nc.scalar.activation(a_buf[s][:, q, :], psa[:], AF.Exp,
                                         accum_out=S_sl[s][:, q:q + 1])
            for s in range(4):
                img = 2 * t + s // 2
                R32 = rbf.tile([128, 32], F16, tag="R32", bufs=4)
                nc.vector.memset(R32[:], 0.0)
                with nc.allow_low_precision(reason="R feeds fp16 matmul"):
                    nc.vector.reciprocal(R32[:, img:32:8], S_sl[s][:])
                for q in range(4):
                    nc.tensor.matmul(
                        ppool[0:8, :],
                        R32[:, 8 * q:8 * q + 8], a_buf[s][:, q, :],
                        start=(n_pool_mm[0] == 0),
                        stop=(n_pool_mm[0] == 63))
                    n_pool_mm[0] += 1

        # ================= MLP =================
        pslim = work.tile([8, 512], F16, tag="pslim")
        with nc.allow_low_precision(reason="pooled to fp16 for MLP"):
            nc.vector.tensor_copy(pslim[:], ppool[0:8, :])
        for q in range(4):
            ptr8 = pc.tile([128, 8], F16, tag="psc", name=f"ptr_{q}")
            nc.tensor.transpose(ptr8[:], pslim[:, 128 * q:128 * q + 128],
                                ident8)
            nc.vector.tensor_copy(mlp_rhs[:, q, :], ptr8[:])
        psz_t = pc.tile([128, 32, 16], F32, tag="psc", name="psz")
        psz = psz_t.rearrange("p a b -> p (a b)")[0:20, 0:B_CORE]
        for q in range(4):
            nc.tensor.matmul(psz[:], m1w[:, q, :], mlp_rhs[:, q, :],
                             start=(q == 0), stop=(q == 3))
        z = work.tile([20, B_CORE], F16, tag="z")
        nc.scalar.activation(z[:], psz[:], AF.Relu, bias=m1b[:])
        pso_t = pc.tile([128, 32, 16], F32, tag="psc", name="pso")
        pso = pso_t.rearrange("p a b -> p (a b)")[0:10, 0:B_CORE]
        nc.tensor.matmul(pso[:], m2w, z[:], start=True, stop=True)
        ot = work.tile([10, B_CORE], F32, tag="ot")
        nc.scalar.activation(ot[:], pso[:], AF.Identity, bias=m2b[:])
        nc.sync.dma_start(outs["out"].rearrange("b o -> o b"), ot[:])


# --------------------------------------------------------------------------
# entry point
# --------------------------------------------------------------------------

_CACHE = {}

IN_SPECS = {
    "rhs1": ([54, B_CORE * H * 16], F16),
    "cpack": ([128, CPACK_COLS], F16),
    "m1b": ([20, 1], F32),
    "m2b": ([10, 1], F32),
}


def get_compiled():
    if "nc" not in _CACHE:
        nc = bacc.Bacc("TRN2", target_bir_lowering=False, debug=False,
                       num_devices=N_CORES)
        ins = {k: nc.dram_tensor(k, shp, dt, kind="ExternalInput").ap()
               for k, (shp, dt) in IN_SPECS.items()}
        outs = {"out": nc.dram_tensor("out", [B_CORE, 10], F32,
                                      kind="ExternalOutput").ap()}
        with tile.TileContext(nc) as tc:
            build_kernel(tc, outs, ins)
        nc.compile()
        _CACHE.update(nc=nc, ins=ins, outs=outs)
    return _CACHE["nc"]


def make_in_maps(x, w1, b1, w2, b2, w3, b3, w4, b4, codebook, sigma,
                 l1_w, l1_b, l2_w, l2_b):
    for b in (b1, b2, b3, b4):
        assert np.abs(np.asarray(b)).max() == 0.0, "conv biases assumed zero"
    cm = _prep_weights(np.asarray(w1, np.float32), np.asarray(w2, np.float32),
                       np.asarray(w3, np.float32), np.asarray(w4, np.float32),
                       np.asarray(codebook, np.float32),
                       np.asarray(sigma, np.float32),
                       np.asarray(l1_w, np.float32),
                       np.asarray(l1_b, np.float32),
                       np.asarray(l2_w, np.float32),
                       np.asarray(l2_b, np.float32))
    x = np.asarray(x, np.float32)
    in_maps = []
    for c in range(N_CORES):
        rhs1 = _prep_rhs1(x[B_CORE * c:B_CORE * (c + 1)]).astype(np.float16)
        m = dict(cm)
        m["rhs1"] = rhs1
        in_maps.append(m)
    return in_maps


def kernel(x, w1, b1, w2, b2, w3, b3, w4, b4, codebook, sigma,
           l1_w, l1_b, l2_w, l2_b):
    nc = get_compiled()
    in_maps = make_in_maps(x, w1, b1, w2, b2, w3, b3, w4, b4, codebook,
                           sigma, l1_w, l1_b, l2_w, l2_b)
    res = bass_utils.run_bass_kernel_spmd(nc, in_maps, list(range(N_CORES)))
    out = np.concatenate([res.results[c]["out"] for c in range(N_CORES)],
                         axis=0)
    return out.astype(np.float32)


# revision 19
# speedup vs baseline: 1.2743x; 1.2743x over previous
"""Trainium2 Bass kernel for nn_Boftrainer_48284022342309 (vq_codebook).

Data-parallel over batch: 64 images -> 8 cores x 8 images each.

Layout conventions (chosen so every SB->SB rhs-build DMA moves
partition-contiguous blocks):
  activation partitions: (px outer, channel inner):  p = px*C + c
  conv rhs partitions:   (j outer, channel inner):   p = j*C + c
  second image of a pair lives at partition offset 64.

Per-core pipeline (everything stays on-chip after the initial loads):
  warmup PE p-state ramp: ~10 dummy matmuls during the input load phase
         bring the tensor engine to full clock before conv1 starts.
  conv1  host-im2col rhs [54=(c,dy,j), (b,y,xb)] fp16; one matmul per
         (img, y-half), M=64=(px4,o16) multipixel columns.
  conv2  rhs [96=(j6,c16), (y66,xb16)] via contiguous SB->SB DMAs;
         3 dy-accumulated matmuls per image, image pair packed in PSUM.
  pool   pool-y: tensor_tensor max on free-strided PSUM views;
         pool-x: stream_shuffle(group half-swap) + fused relu/max.
  conv3  rhs [64=(j4,c16), (Y34,xb16)]; 3 dy matmuls M=48=(px2,o24).
  conv4  rhs [96=(j4,c24), (y34,xb16)]; px-split matmuls M=16 put each
         (img,px) f-slot at a 32-aligned partition base.
  RBF    f' = [f(16); f2; 1] rows per slot; four fp16 matmuls per slot
         against the sigma-augmented codebook fill a 4-bank PSUM strip
         with -sigma*dist2; ONE wide ACT Exp (2048 elems) produces a;
         per-pixel sums S on DVE (reduce over the K axis); reciprocal;
         pooled rows accumulate into a single PSUM bank with image i's
         pooled vector landing on PSUM row i (reciprocal placed at lhsT
         column i).
  MLP    pooled rows 0..8 cast once to SBUF, transposed to [128,(q,b)]
         via PE; two small matmuls; biases via per-partition ACT bias.
"""
import sys
sys.path.insert(0, "/opt/trn_rl_repo")

import contextlib

import numpy as np

import concourse.bass as bass
import concourse.tile as tile
from concourse import bacc, mybir
from concourse import bass_utils

F32 = mybir.dt.float32
F16 = mybir.dt.float16

N_CORES = 8
B_CORE = 8
H = W = 64
HP = 32
KCB = 512

AF = mybir.ActivationFunctionType
ALU = mybir.AluOpType

# column map of the packed fp16 constant buffer [128, CPACK_COLS]
_C_W1P = 0            # [54, 64]
_C_W2P = 64           # [96, 3*64]
_C_W3P = 256          # [64, 3*64]
_C_W4P0 = 448         # [97, 3*32]
_C_W4P1 = 544         # [97, 3*32]
_C_CAUG = 640         # [128, 512]
_C_BLOCKE = 1152      # [128, 128]
_C_M1W = 1280         # [128, 4*20]
_C_M2W = 1360         # [20, 10]
_C_ID8 = 1370         # [8, 8]
CPACK_COLS = 1378


# --------------------------------------------------------------------------
# host-side transforms (numpy)
# --------------------------------------------------------------------------

def _conv_weight_mp(w, P, J):
    """w [O,C,3,3] -> per-dy lhsT [J*C, P*O]:
    W[dy][j*C+c, px*O+o] = w[o,c,dy,j-px] if 0<=j-px<=2 else 0."""
    O, C = w.shape[0], w.shape[1]
    out = np.zeros((3, J * C, P * O), np.float32)
    for dy in range(3):
        for j in range(J):
            for px in range(P):
                dx = j - px
                if 0 <= dx <= 2:
                    out[dy, j * C:(j + 1) * C, px * O:(px + 1) * O] = \
                        w[:, :, dy, dx].T
    return out


def _prep_weights(w1, w2, w3, w4, codebook, sigma, l1_w, l1_b, l2_w, l2_b):
    cp = np.zeros((128, CPACK_COLS), np.float32)
    # conv1: full (c,dy,j) contraction in one matmul: [54, 64=(px4,o16)]
    W1p = np.zeros((3, 3, 6, 4, 16), np.float32)  # c dy j px o
    for j in range(6):
        for px in range(4):
            dx = j - px
            if 0 <= dx <= 2:
                W1p[:, :, j, px, :] = w1[:, :, :, dx].transpose(1, 2, 0)
    cp[0:54, _C_W1P:_C_W1P + 64] = W1p.reshape(54, 64)

    w2p = _conv_weight_mp(w2, P=4, J=6).transpose(1, 0, 2)   # [96,3,64]
    cp[0:96, _C_W2P:_C_W2P + 192] = w2p.reshape(96, 192)
    w3p = _conv_weight_mp(w3, P=2, J=4)                       # [3,64,48]
    w3pad = np.zeros((3, 64, 64), np.float32)
    w3pad[:, :, 0:48] = w3p
    cp[0:64, _C_W3P:_C_W3P + 192] = w3pad.transpose(1, 0, 2).reshape(64, 192)
    w4p = _conv_weight_mp(w4, P=2, J=4)                       # [3,96,32]
    # [97,...]: row 96 pairs with the constant-1 row of rhs4 and writes
    # 1.0 into ps4 row 32s+17 (the "ones" aug row) at dy=1.
    for px in (0, 1):
        wp = np.zeros((97, 3, 32), np.float32)
        wp[0:96, :, 0:16] = w4p[:, :, 16 * px:16 * px + 16].transpose(1, 0, 2)
        wp[96, 1, 17] = 1.0
        base = _C_W4P0 if px == 0 else _C_W4P1
        cp[0:97, base:base + 96] = wp.reshape(97, 96)

    c2 = (codebook * codebook).sum(axis=1)
    caug = np.zeros((32, KCB), np.float32)
    caug[0:16] = (2.0 * sigma[:, None] * codebook).T
    caug[16] = -sigma
    caug[17] = -sigma * c2
    cp[:, _C_CAUG:_C_CAUG + KCB] = np.tile(caug, (4, 1))

    be = np.zeros((128, 128), np.float32)
    for s in range(4):
        be[32 * s:32 * s + 16, 32 * s + 16] = 1.0
    cp[:, _C_BLOCKE:_C_BLOCKE + 128] = be

    l1s = l1_w / float(HP * HP)                               # [20, 512]
    m1w = np.ascontiguousarray(l1s.T).reshape(4, 128, 20)     # (q p) o
    cp[:, _C_M1W:_C_M1W + 80] = m1w.transpose(1, 0, 2).reshape(128, 80)
    cp[0:20, _C_M2W:_C_M2W + 10] = np.ascontiguousarray(l2_w.T)
    cp[0:8, _C_ID8:_C_ID8 + 8] = np.eye(8, dtype=np.float32)

    return {
        "cpack": cp.astype(np.float16),
        "m1b": l1_b.reshape(20, 1).astype(np.float32),
        "m2b": l2_b.reshape(10, 1).astype(np.float32),
    }


def _prep_rhs1(x):
    """x [B,3,64,64] -> [54=(c,dy,j), B*64*16] fp32.
    rhs1[(c,dy,j),(b,y,xb)] = xpad[b, c, y+dy, 4*xb+j] (xpad has 1-px halo)."""
    B = x.shape[0]
    xpad = np.zeros((B, 3, H + 2, W + 2), np.float32)
    xpad[:, :, 1:-1, 1:-1] = x
    rhs1 = np.empty((3, 3, 6, B, H, 16), np.float32)
    xb_idx = 4 * np.arange(16)
    for dy in range(3):
        for j in range(6):
            rhs1[:, dy, j] = xpad[:, :, dy:dy + H, :][:, :, :, xb_idx + j] \
                .transpose(1, 0, 2, 3)
    return rhs1.reshape(54, B * H * 16)


# --------------------------------------------------------------------------
# device kernel
# --------------------------------------------------------------------------

def build_kernel(tc, outs, ins):
    nc = tc.nc
    with contextlib.ExitStack() as ctx:
        consts = ctx.enter_context(tc.tile_pool(name="consts", bufs=1))
        acts = ctx.enter_context(tc.tile_pool(name="acts", bufs=1))
        work = ctx.enter_context(tc.tile_pool(name="work", bufs=3))
        rbf = ctx.enter_context(tc.tile_pool(name="rbf", bufs=2))
        pc = ctx.enter_context(tc.tile_pool(name="pc", bufs=3, space="PSUM"))
        pa = ctx.enter_context(tc.tile_pool(name="pa", bufs=4, space="PSUM"))
        pp = ctx.enter_context(tc.tile_pool(name="pp", bufs=1, space="PSUM"))

        # ---- constants: one big fp16 pack + two tiny f32 biases ----
        cpack = consts.tile([128, CPACK_COLS], F16, tag="cpack", name="cpack")
        nc.sync.dma_start(cpack[:], ins["cpack"][:])
        m1b = consts.tile([20, 1], F32, tag="m1b", name="m1b")
        nc.scalar.dma_start(m1b[:], ins["m1b"][:])
        m2b = consts.tile([10, 1], F32, tag="m2b", name="m2b")
        nc.scalar.dma_start(m2b[:], ins["m2b"][:])

        w1p = cpack[0:54, _C_W1P:_C_W1P + 64]
        w2p = cpack[0:96, _C_W2P:_C_W2P + 192].rearrange(
            "p (d m) -> p d m", d=3)
        w3p = cpack[0:64, _C_W3P:_C_W3P + 192].rearrange(
            "p (d m) -> p d m", d=3)
        w4p0 = cpack[0:97, _C_W4P0:_C_W4P0 + 96].rearrange(
            "p (d m) -> p d m", d=3)
        w4p1 = cpack[0:97, _C_W4P1:_C_W4P1 + 96].rearrange(
            "p (d m) -> p d m", d=3)
        caug = cpack[:, _C_CAUG:_C_CAUG + KCB]
        blocke = cpack[:, _C_BLOCKE:_C_BLOCKE + 128]
        m1w = cpack[:, _C_M1W:_C_M1W + 80].rearrange("p (q o) -> p q o", q=4)
        m2w = cpack[0:20, _C_M2W:_C_M2W + 10]
        ident8 = cpack[0:8, _C_ID8:_C_ID8 + 8]

        # ---- input: im2col'd rhs1, 4 chunks on different queues ----
        rhs1 = consts.tile([54, B_CORE, H, 16], F16, tag="rhs1", name="rhs1")
        r1src = ins["rhs1"].rearrange("k (b y xb) -> k b y xb",
                                      b=B_CORE, y=H, xb=16)
        r1eng = [nc.gpsimd, nc.scalar, nc.sync, nc.gpsimd]
        for c in range(4):
            r1eng[c].dma_start(rhs1[:, 2 * c:2 * c + 2], r1src[:, 2 * c:2 * c + 2])

        # ---- PE p-state warm-up: dependency-free dummy matmuls across 4
        # rotating PSUM banks during the load phase (same conditions as the
        # microbenchmark that reaches the full 2.4 GHz p-state).
        wsrc = consts.tile([128, 512], F16, tag="wsrc", name="wsrc")
        nc.vector.memset(wsrc[:], 0.25)
        warm_banks = [pa.tile([128, 512], F32, tag="pa", name=f"warm{_i}")
                      for _i in range(4)]
        for k in range(14):
            nc.tensor.matmul(warm_banks[k % 4][:], wsrc[:, 0:128], wsrc[:],
                             start=True, stop=True)

        def heartbeat(k):
            # one warm matmul to keep the PE activity monitor from
            # re-throttling across a known multi-us PE-idle stretch
            hb = pa.tile([128, 512], F32, tag="pa", name=f"hb{k}")
            nc.tensor.matmul(hb[:], wsrc[:, 0:128], wsrc[:],
                             start=True, stop=True)

        # round-robin DMA queue assignment for the conv rhs builds
        dma_engs = [nc.sync, nc.scalar, nc.gpsimd]
        _dma_i = [0]

        def next_eng():
            e = dma_engs[_dma_i[0] % len(dma_engs)]
            _dma_i[0] += 1
            return e

        # ---- persistent activation buffers ----
        act1 = [acts.tile([128, H, 16], F16, tag=f"act1_{p}", name=f"act1_{p}")
                for p in range(4)]
        rhs2 = [acts.tile([96, H + 2, 16], F16, tag=f"rhs2_{i}", name=f"rhs2_{i}")
                for i in range(4)]
        act2p = [acts.tile([128, HP + 2, 16], F16, tag=f"act2p_{p}",
                           name=f"act2p_{p}") for p in range(4)]
        rhs3 = [acts.tile([64, HP + 2, 16], F16, tag=f"rhs3_{i}", name=f"rhs3_{i}")
                for i in range(4)]
        act3 = [acts.tile([128, HP + 2, 16], F16, tag=f"act3_{p}",
                          name=f"act3_{p}") for p in range(4)]
        rhs4 = [acts.tile([97, HP + 2, 16], F16, tag=f"rhs4_{i}", name=f"rhs4_{i}")
                for i in range(4)]
        f_buf = [acts.tile([128, 512], F16, tag=f"f_{t}", name=f"f_{t}")
                 for t in range(4)]
        mlp_rhs = acts.tile([128, 4, B_CORE], F16, tag="mlp_rhs")

        for r2 in rhs2:
            nc.vector.memset(r2[:, 0:1, :], 0.0)
            nc.vector.memset(r2[:, 65:66, :], 0.0)
            nc.vector.memset(r2[0:16, :, 0:1], 0.0)
            nc.vector.memset(r2[64:96, :, 15:16], 0.0)
        for r3 in rhs3:
            nc.vector.memset(r3[0:16, :, 0:1], 0.0)
            nc.vector.memset(r3[32:64, :, 15:16], 0.0)
        for r4 in rhs4:
            nc.vector.memset(r4[0:24, :, 0:1], 0.0)
            nc.vector.memset(r4[64:96, :, 15:16], 0.0)
            nc.vector.memset(r4[96:97, :, :], 1.0)
        for a2 in act2p:
            nc.vector.memset(a2[:, 0:1, :], 0.0)
            nc.vector.memset(a2[:, 33:34, :], 0.0)
        for a3 in act3:
            nc.vector.memset(a3[:, 0:1, :], 0.0)
            nc.vector.memset(a3[:, 33:34, :], 0.0)

        # ================= conv1 =================
        # act1 partition = 64*half + px*16 + o
        for pair in range(4):
            bA, bB = 2 * pair, 2 * pair + 1
            for h in range(2):
                ps = pc.tile([128, 32, 16], F32, tag="psc")
                nc.tensor.matmul(ps[0:64], w1p,
                                 rhs1[:, bA, 32 * h:32 * h + 32, :],
                                 start=True, stop=True)
                nc.tensor.matmul(ps[64:128], w1p,
                                 rhs1[:, bB, 32 * h:32 * h + 32, :],
                                 start=True, stop=True)
                nc.vector.tensor_scalar_max(
                    act1[pair][:, 32 * h:32 * h + 32, :], ps[:], 0.0)
            heartbeat(12 + pair)

        # ================= conv2 + pool =================
        # j -> (source px, xb shift): x = 4*xb_dst + j - 1 = 4*xb_src + px
        J2 = [(3, -1), (0, 0), (1, 0), (2, 0), (3, 0), (0, 1)]
        HSWAP = [(i + 16) % 32 for i in range(32)]  # swap px-pair halves
        for pair in range(4):
            for half in range(2):
                ioff = 64 * half
                r2 = rhs2[2 * (pair % 2) + half]
                # j=1..4 read contiguous px-blocks 0..3 -> one DMA
                next_eng().dma_start(r2[16:80, 1:65, :],
                                     act1[pair][ioff:ioff + 64, :, :])
                for j in (0, 5):
                    pj, sh = J2[j]
                    n = 16 - abs(sh)
                    d0, s0 = max(0, -sh), max(0, sh)
                    next_eng().dma_start(
                        r2[16 * j:16 * j + 16, 1:65, d0:d0 + n],
                        act1[pair][ioff + 16 * pj:ioff + 16 * pj + 16, :,
                                   s0:s0 + n])
            for h in range(2):
                ps = pc.tile([128, 32, 16], F32, tag="psc")
                for half in range(2):
                    for dy in range(3):
                        nc.tensor.matmul(
                            ps[64 * half:64 * half + 64],
                            w2p[:, dy, :],
                            rhs2[2 * (pair % 2) + half][:, 32 * h + dy:32 * h + dy + 32, :],
                            start=(dy == 0), stop=(dy == 2),
                            tile_position=(0, 64 * half))
                # relu (psum fp32 -> sbuf fp16), then pool on SBUF
                t0 = work.tile([128, 32, 16], F16, tag="t0")
                nc.vector.tensor_scalar_max(t0[:], ps[:], 0.0)
                tp = work.tile([128, 16, 16], F16, tag="tp")
                v = t0[:].rearrange("p (Y yp) x -> p Y yp x", yp=2)
                nc.vector.tensor_tensor(tp[:], v[:, :, 0, :], v[:, :, 1, :],
                                        op=ALU.max)
                # pool-x: swap 16-blocks within 32-groups
                sh_t = work.tile([128, 16, 16], F16, tag="sh")
                nc.vector.stream_shuffle(sh_t[:], tp[:], HSWAP)
                nc.vector.tensor_tensor(
                    act2p[pair][:, 1 + 16 * h:17 + 16 * h, :],
                    sh_t[:], tp[:], op=ALU.max)
                heartbeat(2 * pair + h)

        # ================= conv3 =================
        # x3 = 2*xb3 + j - 1; source px-representative block in {0, 2}
        J3 = [(2, -1), (0, 0), (2, 0), (0, 1)]
        for pair in range(4):
            for half in range(2):
                ioff = 64 * half
                r3 = rhs3[2 * (pair % 2) + half]
                for j in (1, 2):
                    pj, sh = J3[j]
                    next_eng().dma_start(
                        r3[16 * j:16 * j + 16, :, :],
                        act2p[pair][ioff + 16 * pj:ioff + 16 * pj + 16, :, :])
                for j in (0, 3):
                    pj, sh = J3[j]
                    n = 16 - abs(sh)
                    d0, s0 = max(0, -sh), max(0, sh)
                    next_eng().dma_start(
                        r3[16 * j:16 * j + 16, :, d0:d0 + n],
                        act2p[pair][ioff + 16 * pj:ioff + 16 * pj + 16, :,
                                    s0:s0 + n])
            ps = pc.tile([128, 32, 16], F32, tag="psc")
            for half in range(2):
                for dy in range(3):
                    nc.tensor.matmul(
                        ps[64 * half:64 * half + 64],
                        w3p[:, dy, :],
                        rhs3[2 * (pair % 2) + half][:, dy:dy + 32, :],
                        start=(dy == 0), stop=(dy == 2),
                        tile_position=(0, 64 * half))
            nc.vector.tensor_scalar_max(act3[pair][:, 1:33, :], ps[:], 0.0)
            heartbeat(8 + pair)

        # ================= conv4 + f' assembly + RBF =================
        # act3 partition = 64*half + px*24 + o (px in {0,1})
        # RBF is interleaved per image-pair t so the ACT-bound exp work of
        # pair t overlaps the PE/DVE-bound conv4 work of pair t+1.
        # pooled vectors for all 8 images accumulate into ONE psum bank;
        # image i's pooled vector lands on psum row i because its 1/S
        # values sit at lhsT column i.
        J4 = [(1, -1), (0, 0), (1, 0), (0, 1)]
        ppool = pp.tile([128, 512], F32, tag="ppool", name="ppool")
        n_pool_mm = [0]

        for t in range(4):
            fb = f_buf[t]
            ps4 = pc.tile([128, 512], F32, tag="psc", name=f"ps4_{t}")
            for i in range(2):
                img = 2 * t + i
                pair, half = img // 2, img % 2
                ioff = 64 * half
                r4 = rhs4[2 * (t % 2) + i]
                # j=1,2 read contiguous px-blocks 0,1 -> one DMA
                next_eng().dma_start(r4[24:72, :, :],
                                     act3[pair][ioff:ioff + 48, :, :])
                for j in (0, 3):
                    pj, sh = J4[j]
                    n = 16 - abs(sh)
                    d0, s0 = max(0, -sh), max(0, sh)
                    next_eng().dma_start(
                        r4[24 * j:24 * j + 24, :, d0:d0 + n],
                        act3[pair][ioff + 24 * pj:ioff + 24 * pj + 24, :,
                                   s0:s0 + n])
                for px in range(2):
                    s = 2 * i + px
                    w4 = w4p0 if px == 0 else w4p1
                    for dy in range(3):
                        nc.tensor.matmul(
                            ps4[32 * s:32 * s + 32, :],
                            w4[:, dy, :],
                            r4[:, dy:dy + 32, :].rearrange("p y x -> p (y x)"),
                            start=(dy == 0), stop=(dy == 2),
                            tile_position=(0, 32 * s))
            nc.vector.tensor_scalar_max(fb[:], ps4[:], 0.0)
            fsq = work.tile([128, 512], F16, tag="fsq")
            nc.vector.tensor_mul(fsq[:], fb[:], fb[:])
            psf = pc.tile([128, 512], F32, tag="psc", name=f"psf_{t}")
            nc.tensor.matmul(psf[:], blocke, fsq[:], start=True, stop=True)
            nc.vector.tensor_tensor(fb[:], fb[:], psf[:], op=ALU.max)

            # ---- RBF for this image pair ----
            # q-major emission: the 4 slots' psa matmuls of each q batch
            # target distinct 32-row groups and distinct PSUM banks, so the
            # PE runs them nearly concurrently while ACT drains the exps.
            a_buf = [rbf.tile([128, 4, 512], F16, tag="a", name=f"a_{t}_{s}",
                              bufs=8) for s in range(4)]
            S_sl = [rbf.tile([128, 4], F32, tag="S", name=f"S_{t}_{s}",
                             bufs=8) for s in range(4)]
            for q in range(4):
                for s in range(4):
                    psa = pa.tile([128, 512], F32, tag="pa")
                    nc.tensor.matmul(
                        psa[:],
                        fb[32 * s:32 * s + 18, 128 * q:128 * q + 128],
                        caug[32 * s:32 * s + 18, :],
                        start=True, stop=True,
                        tile_position=(32 * s, 0))
                    nc.scalar.activation(a_buf[s][:, q, :], psa[:], AF.Exp,
                                         accum_out=S_sl[s][:, q:q + 1])
            for s in range(4):
                img = 2 * t + s // 2
                R32 = rbf.tile([128, 32], F16, tag="R32", bufs=4)
                nc.vector.memset(R32[:], 0.0)
                with nc.allow_low_precision(reason="R feeds fp16 matmul"):
                    nc.vector.reciprocal(R32[:, img:32:8], S_sl[s][:])
                for q in range(4):
                    nc.tensor.matmul(
                        ppool[0:8, :],
                        R32[:, 8 * q:8 * q + 8], a_buf[s][:, q, :],
                        start=(n_pool_mm[0] == 0),
                        stop=(n_pool_mm[0] == 63))
                    n_pool_mm[0] += 1

        # ================= MLP =================
        pslim = work.tile([8, 512], F16, tag="pslim")
        with nc.allow_low_precision(reason="pooled to fp16 for MLP"):
            nc.vector.tensor_copy(pslim[:], ppool[0:8, :])
        for q in range(4):
            ptr8 = pc.tile([128, 8], F16, tag="psc", name=f"ptr_{q}")
            nc.tensor.transpose(ptr8[:], pslim[:, 128 * q:128 * q + 128],
                                ident8)
            nc.vector.tensor_copy(mlp_rhs[:, q, :], ptr8[:])
        psz_t = pc.tile([128, 32, 16], F32, tag="psc", name="psz")
        psz = psz_t.rearrange("p a b -> p (a b)")[0:20, 0:B_CORE]
        for q in range(4):
            nc.tensor.matmul(psz[:], m1w[:, q, :], mlp_rhs[:, q, :],
                             start=(q == 0), stop=(q == 3))
        z = work.tile([20, B_CORE], F16, tag="z")
        nc.scalar.activation(z[:], psz[:], AF.Relu, bias=m1b[:])
        pso_t = pc.tile([128, 32, 16], F32, tag="psc", name="pso")
        pso = pso_t.rearrange("p a b -> p (a b)")[0:10, 0:B_CORE]
        nc.tensor.matmul(pso[:], m2w, z[:], start=True, stop=True)
        ot = work.tile([10, B_CORE], F32, tag="ot")
        nc.scalar.activation(ot[:], pso[:], AF.Identity, bias=m2b[:])
        nc.sync.dma_start(outs["out"].rearrange("b o -> o b"), ot[:])


# --------------------------------------------------------------------------
# entry point
# --------------------------------------------------------------------------

_CACHE = {}

IN_SPECS = {
    "rhs1": ([54, B_CORE * H * 16], F16),
    "cpack": ([128, CPACK_COLS], F16),
    "m1b": ([20, 1], F32),
    "m2b": ([10, 1], F32),
}


def get_compiled():
    if "nc" not in _CACHE:
        nc = bacc.Bacc("TRN2", target_bir_lowering=False, debug=False,
                       num_devices=N_CORES)
        ins = {k: nc.dram_tensor(k, shp, dt, kind="ExternalInput").ap()
               for k, (shp, dt) in IN_SPECS.items()}
        outs = {"out": nc.dram_tensor("out", [B_CORE, 10], F32,
                                      kind="ExternalOutput").ap()}
        with tile.TileContext(nc) as tc:
            build_kernel(tc, outs, ins)
        nc.compile()
        _CACHE.update(nc=nc, ins=ins, outs=outs)
    return _CACHE["nc"]


def make_in_maps(x, w1, b1, w2, b2, w3, b3, w4, b4, codebook, sigma,
                 l1_w, l1_b, l2_w, l2_b):
    for b in (b1, b2, b3, b4):
        assert np.abs(np.asarray(b)).max() == 0.0, "conv biases assumed zero"
    cm = _prep_weights(np.asarray(w1, np.float32), np.asarray(w2, np.float32),
                       np.asarray(w3, np.float32), np.asarray(w4, np.float32),
                       np.asarray(codebook, np.float32),
                       np.asarray(sigma, np.float32),
                       np.asarray(l1_w, np.float32),
                       np.asarray(l1_b, np.float32),
                       np.asarray(l2_w, np.float32),
                       np.asarray(l2_b, np.float32))
    x = np.asarray(x, np.float32)
    in_maps = []
    for c in range(N_CORES):
        rhs1 = _prep_rhs1(x[B_CORE * c:B_CORE * (c + 1)]).astype(np.float16)
        m = dict(cm)
        m["rhs1"] = rhs1
        in_maps.append(m)
    return in_maps


def kernel(x, w1, b1, w2, b2, w3, b3, w4, b4, codebook, sigma,
           l1_w, l1_b, l2_w, l2_b):
    nc = get_compiled()
    in_maps = make_in_maps(x, w1, b1, w2, b2, w3, b3, w4, b4, codebook,
                           sigma, l1_w, l1_b, l2_w, l2_b)
    res = bass_utils.run_bass_kernel_spmd(nc, in_maps, list(range(N_CORES)))
    out = np.concatenate([res.results[c]["out"] for c in range(N_CORES)],
                         axis=0)
    return out.astype(np.float32)
